# revision 32
# baseline (speedup 1.0000x reference)
"""AngularPenaltySMLoss (CosFace, s=20, m=0) on 8 TRN2 NeuronCores.

With m=0 the reference loss algebraically reduces to
    loss_i = s*wf[i, l_i] - log(sum_j exp(s*wf[i, j]))
    out    = -mean_i(loss_i)
(denominator = exp(s*t) + (rowsum - exp(s*t)) = rowsum exactly).

Data-parallel: core c owns rows [c*1024, (c+1)*1024). Per core:
  - stream the [1024, 32000] f32 shard through SBUF in [128, 4000] chunks
    (DMA-bound at the ~358 GB/s per-core HBM rate); ScalarE
    activation(Exp, scale=20) with accum_out produces per-chunk row sums
    (fused exp + row-reduce, one instruction per chunk),
  - gather wf[i, l_i] on-device with indirect_dma_start (one flat int32
    element offset per partition, precomputed on host from labels),
  - reduce 20*t - log(rowsum) over the shard to one scalar on-device.
Host sums the 8 partial scalars: out = -(sum of partials)/8192.
"""

import numpy as np

import concourse.bacc as bacc
import concourse.bass as bass
import concourse.bass_isa as bass_isa
import concourse.tile as tile
from concourse import mybir
from concourse.bass import _bass_rust
from concourse.bass_utils import run_bass_kernel_spmd

_DEP_NOSYNC = _bass_rust.DependencyInfo(sync=False, no_sync=True)

B, C = 8192, 32000
NCORES = 8
B_SH = B // NCORES      # 1024 rows per core
P = 128                 # partitions
G = B_SH // P           # 8 row groups per core
T = 4000                # column chunk (2.1 MB per DMA: big enough for
                        # near-peak HBM rate, small enough that the 8-deep
                        # ring fits SBUF and the tail ACT stays short)
NCH = C // T            # 8 chunks per row group
S = 20.0

TRACE = False           # optional NTFF profiling (needs antenv.axon_hooks)
LAST_EXEC_NS = None

_NC_CACHE = {}


def _build():
    f32 = mybir.dt.float32
    i32 = mybir.dt.int32

    nc = bacc.Bacc()
    wf_d = nc.declare_dram_parameter("wf", [B_SH, C], f32, isOutput=False)
    # offs[p, g] = (g*128 + p)*C + labels[g*128 + p] -- flat element offset
    # of each row's target entry in the wf shard (exact int32 from host).
    off_d = nc.declare_dram_parameter("offs", [P, G], i32, isOutput=False)
    out_d = nc.declare_dram_parameter("out", [1, 1], f32, isOutput=True)

    with tile.TileContext(nc) as tc:
        with tc.tile_pool(name="small", bufs=1) as sm_pool:
            # ---- gather wf[i, l_i] via indirect DMA --------------------
            offs = sm_pool.tile([P, G], i32)
            nc.sync.dma_start(out=offs[:], in_=off_d[:, :])

            # t_raw[p, g] = wf_flat[offs[p, g]].  The HW indirect DMA applies
            # ONE offset per partition and copies out.shape[1] consecutive
            # elements, so gather one column per call.
            t_raw = sm_pool.tile([P, G], f32)
            for g in range(G):
                nc.gpsimd.indirect_dma_start(
                    out=t_raw[:, g : g + 1],
                    out_offset=None,
                    in_=wf_d[:, :],
                    in_offset=bass.IndirectOffsetOnAxis(
                        ap=offs[:, g : g + 1], axis=1
                    ),
                    element_offset=0,
                )

            # ---- streaming exp row sums --------------------------------
            # All ACTs dump their elementwise output into one shared scratch
            # (only accum_out matters).  The ACT->ACT WAW dep on scratch is
            # demoted to a nosync (program-order) dep: the ACT pipeline
            # executes in order, scratch is never read, and the accum
            # columns are disjoint -- each ACT then carries exactly one
            # semaphore wait (its input DMA).  bias=0.0 resolves to the
            # pre-barrier const AP (no dep).
            # Input tiles are an explicit 8-deep ping-pong ring rather than
            # a tile_pool: pool slot recycling injects release fences onto
            # the DMAs beyond the 1-wait DMA ISA struct budget.  The ring's
            # DMA->DMA WAW dep (chunk k over chunk k-8, same tensor) is
            # demoted to nosync: k and k-8 share queue parity, so both ride
            # the same HWDGE FIFO and each partition's bytes go through the
            # same SDMA engine -- the rewrite is ordered in hardware.  The
            # WAR on the reader ACT of chunk k-8 stays as the DMA's single
            # semaphore wait.
            rs_parts = sm_pool.tile([P, G * NCH], f32)
            scratch = sm_pool.tile([P, T], f32)
            ring = [
                sm_pool.tile([P, T], f32, name=f"in{j}", tag=f"in{j}")
                for j in range(NCH)
            ]
            ring_dma = [None] * NCH
            prev_act = None
            for g in range(G):
                for ci in range(NCH):
                    k = g * NCH + ci
                    tile_in = ring[k % NCH]
                    # alternate the two physical HWDGE rings (SP / ACT) so
                    # DMA issue and completion handling overlap
                    qeng = nc.sync if k % 2 == 0 else nc.scalar
                    dma = qeng.dma_start(
                        out=tile_in[:],
                        in_=wf_d[g * P : (g + 1) * P, ci * T : (ci + 1) * T],
                    ).ins
                    if ring_dma[k % NCH] is not None:
                        prev_dma = ring_dma[k % NCH]
                        dma.try_remove_dependency(prev_dma.name)
                        dma.add_dependency(prev_dma.name, _DEP_NOSYNC)
                    ring_dma[k % NCH] = dma
                    act = nc.scalar.activation(
                        out=scratch[:],
                        in_=tile_in[:],
                        func=mybir.ActivationFunctionType.Exp,
                        scale=S,
                        accum_out=rs_parts[:, k : k + 1],
                    ).ins
                    if prev_act is not None:
                        act.try_remove_dependency(prev_act.name)
                        act.add_dependency(prev_act.name, _DEP_NOSYNC)
                    prev_act = act

            # ---- epilogue ----------------------------------------------
            rs_tot = sm_pool.tile([P, G], f32)
            for g in range(G):
                nc.vector.tensor_reduce(
                    out=rs_tot[:, g : g + 1],
                    in_=rs_parts[:, g * NCH : (g + 1) * NCH],
                    axis=mybir.AxisListType.X,
                    op=mybir.AluOpType.add,
                )
            # t20 = S*t_raw and loga = Ln(rs_tot) both on the ACT engine, so
            # the DVE combine below has a single cross-engine dependency
            # (one semaphore wait covers both ACT-produced operands).
            t20 = sm_pool.tile([P, G], f32)
            nc.scalar.activation(
                out=t20[:],
                in_=t_raw[:],
                func=mybir.ActivationFunctionType.Copy,
                scale=S,
            )
            loga = sm_pool.tile([P, G], f32)
            nc.scalar.activation(
                out=loga[:],
                in_=rs_tot[:],
                func=mybir.ActivationFunctionType.Ln,
            )
            tmp = sm_pool.tile([P, G], f32)
            loss_acc = sm_pool.tile([P, 1], f32)
            nc.vector.scalar_tensor_tensor(
                out=tmp[:],
                in0=t20[:],
                scalar=1.0,
                in1=loga[:],
                op0=mybir.AluOpType.mult,
                op1=mybir.AluOpType.subtract,
                accum_out=loss_acc[:],
            )
            total = sm_pool.tile([P, 1], f32)
            nc.gpsimd.partition_all_reduce(
                total[:], loss_acc[:], channels=P, reduce_op=bass_isa.ReduceOp.add
            )
            nc.sync.dma_start(out=out_d[:, :], in_=total[0:1, :])

    nc.finalize()
    return nc


def _get_nc():
    if "nc" not in _NC_CACHE:
        _NC_CACHE["nc"] = _build()
    return _NC_CACHE["nc"]


def kernel(wf, labels):
    global LAST_EXEC_NS
    wf = np.asarray(wf, dtype=np.float32)
    labels = np.asarray(labels).astype(np.int64)
    assert wf.shape == (B, C) and labels.shape == (B,)

    nc = _get_nc()
    in_maps = []
    for c in range(NCORES):
        wf_sh = np.ascontiguousarray(wf[c * B_SH : (c + 1) * B_SH])
        lab_sh = labels[c * B_SH : (c + 1) * B_SH]
        # offs[p, g] = (g*128 + p)*C + labels[g*128 + p]
        rows = np.arange(B_SH, dtype=np.int64).reshape(G, P).T * C
        offs = (rows + lab_sh.reshape(G, P).T).astype(np.int32)
        in_maps.append({"wf": wf_sh, "offs": np.ascontiguousarray(offs)})

    res = run_bass_kernel_spmd(
        nc, in_maps, core_ids=list(range(NCORES)), trace=TRACE
    )
    LAST_EXEC_NS = res.exec_time_ns
    total = sum(float(r["out"][0, 0]) for r in res.results)
    return np.asarray(-(total / B), dtype=np.float32)
